# revision 1
# baseline (speedup 1.0000x reference)
"""TRN2 Bass kernel for nn_Decoder_SICA (dense CNN decoder), 8-core data parallel.

Network (per sample):
  stage0: per-sample grouped conv_transpose (stride==kernel) == block einsum
          A(512,2,2) x S(64,3,5,5) -> x0 (384,10,10)
  conv1:  384->384 3x3 pad1        -> (384,10,10) relu
  conv2:  384->384 3x3 s2 pad1     -> (384,5,5)   relu
  conv3:  384->512 3x3 pad1        -> (512,5,5)   relu
  conv4:  512->512 5x5 valid       -> (512,1,1)   relu
  linear: 512->10

Design notes:
  - batch dim sharded 8 ways (64 samples/core), weights replicated.
  - activations live in SBUF as [C_part=128, (c_tile), H, W, B] - batch
    INNERMOST.
  - convs run as accumulation groups of per-offset matmuls with
    boundary-split output rectangles (no padding, no im2col copies); PSUM's
    per-element has_written bit makes partial-coverage accumulation exact.
  - everything bf16 on the PE: bf16 stationaries get Fast-Weight-Load
    (LDWEIGHTS 53ns vs 107ns for fp32/fp32r), so the per-matmul weight load
    fully hides under the >=256-col rhs stream. End-to-end rel err ~5e-3.
  - all conv weights live in ONE 16-slot SBUF pool; w2/w3/w4 prefetch on the
    ACT HWDGE ring (separate FIFO from the sync ring carrying stage0
    staging), so conv4's 13MB of weights stream during conv1-3 instead of
    stalling conv4.
  - stage0's per-sample contraction (g,c)=64 is made a single K=64 matmul
    per (h,w) via a host-built block-diagonal stationary matrix from A.
"""

import numpy as np
import ml_dtypes

import concourse.bacc as bacc
import concourse.mybir as mybir
from concourse.tile import TileContext
from concourse.bass_utils import run_bass_kernel_spmd

P = 128
B_FULL = 512
NCORES = 8
BL = B_FULL // NCORES        # 64 samples per core
SLAB = 32                    # stage0+conv1 batch slab
SG0 = 8                      # stage0 samples per staging tile (one DMA each)
BF16 = mybir.dt.bfloat16
FP32 = mybir.dt.float32


def _build_program(loop_n=1, accum_out=False):
    nc = bacc.Bacc("TRN2", target_bir_lowering=False, debug=False,
                   num_devices=NCORES)

    # ---- DRAM I/O (per core) ----
    # stg packs, per sample, the two hw-pair block-diag A stationaries
    # (2*128 cols over 128 contraction rows) and the block-diag S moving
    # operand (150 cols): one DMA per 8-sample staging tile.
    stg = nc.dram_tensor("stg", [BL // SG0, P, SG0, 2 * P + 150], BF16,
                         kind="ExternalInput")
    w1t = nc.dram_tensor("w1t", [3, P, 3, 3, 3, P], BF16, kind="ExternalInput")
    w2t = nc.dram_tensor("w2t", [3, P, 3, 3, 3, P], BF16, kind="ExternalInput")
    w3t = nc.dram_tensor("w3t", [4, P, 3, 3, 3, P], BF16, kind="ExternalInput")
    w4t = nc.dram_tensor("w4t", [4, P, 4, 5, 5, P], BF16, kind="ExternalInput")
    wl = nc.dram_tensor("wl", [P, 4, 10], BF16, kind="ExternalInput")
    biases = nc.dram_tensor("biases", [P, 14], FP32, kind="ExternalInput")
    blrep = nc.dram_tensor("blrep", [BL, 10], FP32, kind="ExternalInput")
    out_d = nc.dram_tensor("OUT", [BL, 10], FP32, kind="ExternalOutput")
    # bias columns: b1 -> 0:3, b2 -> 3:6, b3 -> 6:10, b4 -> 10:14

    RELU = mybir.ActivationFunctionType.Relu

    with TileContext(nc) as tc:
        with (
            tc.tile_pool(name="acts_a", bufs=2) as pool_a,   # x0 slabs, then x2
            tc.tile_pool(name="acts_b", bufs=1) as pool_b,   # x1, then x3
            tc.tile_pool(name="wpool", bufs=15) as wpool,    # all conv weights
            tc.tile_pool(name="s0pool", bufs=3) as s0pool,   # stage0 A/S staging
            tc.tile_pool(name="misc", bufs=1) as misc,
            tc.tile_pool(name="pspool", bufs=3, space="PSUM") as pspool,
            tc.tile_pool(name="ps0pool", bufs=2, space="PSUM") as ps0pool,
            tc.tile_pool(name="pslin", bufs=1, space="PSUM") as pslin,
        ):
            # small constants ride the ACT HWDGE ring; the sync ring is
            # reserved for stage0 staging (the startup critical path).
            bias_t = misc.tile([P, 14], FP32, name="bias_t")
            nc.gpsimd.dma_start(bias_t[:], biases[:])
            wl_t = misc.tile([P, 4, 10], BF16, name="wl_t")
            nc.gpsimd.dma_start(wl_t[:], wl[:])
            bl_t = misc.tile([BL, 10], FP32, name="bl_t")
            nc.gpsimd.dma_start(bl_t[:], blrep[:])

            def body():
                _emit_body(nc, tc, pool_a, pool_b, wpool, s0pool, misc,
                           pspool, ps0pool, pslin, bias_t, wl_t, bl_t,
                           stg, w1t, w2t, w3t, w4t, out_d, RELU,
                           accum_out)

            if loop_n > 1:
                with tc.For_i(0, loop_n, 1):
                    body()
            else:
                body()

    nc.compile()
    return nc


def _emit_body(nc, tc, pool_a, pool_b, wpool, s0pool, misc, pspool, ps0pool,
               pslin, bias_t, wl_t, bl_t, stg, w1t, w2t, w3t, w4t, out_d,
               RELU, accum_out=False):
    x1 = pool_b.tile([P, 3, 10, 10, BL], BF16, name="x1", tag="bufB")

    # ---- weight prefetch (all on the gpsimd SWDGE queue: the ACT/SP
    # sequencers stay free for evictions and staging) ----
    w1_tiles = []
    for cot in range(3):
        w_t = wpool.tile([P, 3, 3, 3, P], BF16, name=f"w1_{cot}", tag="w")
        nc.gpsimd.dma_start(w_t[:], w1t[cot])
        w1_tiles.append(w_t)
    w2_tiles = []
    for cot in range(3):
        w_t = wpool.tile([P, 3, 3, 3, P], BF16, name=f"w2_{cot}", tag="w")
        nc.gpsimd.dma_start(w_t[:], w2t[cot])
        w2_tiles.append(w_t)
    w3_tiles = []
    for cot in range(4):
        w_t = wpool.tile([P, 3, 3, 3, P], BF16, name=f"w3_{cot}", tag="w")
        nc.gpsimd.dma_start(w_t[:], w3t[cot])
        w3_tiles.append(w_t)
    w4_tiles = {}
    w4_order = [(cot, cit) for cot in range(4) for cit in range(4)]

    def w4_prefetch(n):
        for cot, cit in w4_order[len(w4_tiles):len(w4_tiles) + n]:
            w_t = wpool.tile([P, 5, 5, P], BF16, name=f"w4_{cot}_{cit}",
                             tag="w")
            nc.gpsimd.dma_start(w_t[:], w4t[cot, :, cit])
            w4_tiles[(cot, cit)] = w_t

    w4_prefetch(5)  # fills the 15-slot pool alongside w1/w2/w3

    # ---- per-slab: stage0 + conv1 ----
    # Slab widths (8, 24, 32): conv1 starts once the first 8 samples are
    # evicted. The PE executes in emission order, so the NEXT slab's stage0
    # pairs are hand-interleaved between conv1 row-groups of the current
    # slab: the PE chews conv1 matmuls while the pair evictions (DVE)
    # complete in the shadow.
    SLABS = [(0, 8), (8, 24), (32, 32)]

    def stage0_tasks(slab, b0, W, x0s):
        """Closures, each emitting one 2-sample psum pair (+ staging DMA at
        group boundaries) and its eviction."""
        st_tiles = {}
        tasks = []
        for bg in range(W // SG0):
            b = b0 + bg * SG0
            for pj in range(SG0 // 2):
                def task(bg=bg, b=b, pj=pj):
                    if pj == 0:
                        st = s0pool.tile([P, SG0, 2 * P + 150], BF16,
                                         name=f"st{b}", tag="st")
                        if slab == 0 and bg == 0:
                            # split the very first staging transfer so the
                            # first pair's matmuls start ~1us earlier
                            nc.sync.dma_start(st[:, 0:2], stg[b // SG0, :, 0:2])
                            nc.sync.dma_start(st[:, 2:], stg[b // SG0, :, 2:])
                        else:
                            nc.sync.dma_start(st[:], stg[b // SG0])
                        st_tiles[bg] = st
                    st = st_tiles[bg]
                    ps0 = ps0pool.tile([P, 2, 300], FP32,
                                       padded_shape=[P, 2, 512],
                                       name=f"ps0_{b + 2 * pj}", tag="ps0")
                    ps0v = ps0.rearrange("p s (o r c) -> p s o r c",
                                         o=3, r=10, c=10)
                    # the staged S columns are host-permuted to (o, p, w, q)
                    # order, so hw-pair matmul j writes rows 5j..5j+5 of the
                    # (o, r, c) psum image as a plain 3-D slice
                    for si in range(2):
                        bi = 2 * pj + si
                        for j in range(2):
                            nc.tensor.matmul(
                                ps0v[:, si, :, 5 * j : 5 * j + 5, :],
                                lhsT=st[:, bi, j * P : (j + 1) * P],
                                rhs=st[:, bi, 2 * P :],
                                start=(j == 0),
                                stop=(j == 1),
                            )
                    sb0 = bg * SG0 + 2 * pj
                    # iterate (sample, o, r, c): src reads each psum slot
                    # contiguously; dst is batch-innermost SBUF layout.
                    src = ps0[:, :, :]
                    dst = x0s[:, :, :, :, sb0 : sb0 + 2].transpose(
                        [0, 4, 1, 2, 3])
                    if slab == 0 and pj % 2 == 1:
                        nc.scalar.copy(dst, src)
                    else:
                        nc.vector.tensor_scalar_add(dst, src, 0.0)
                tasks.append(task)
        return tasks

    x0_tiles = []
    for slab, (b0, W) in enumerate(SLABS):
        x0_tiles.append(pool_a.tile(
            [P, 3, 10, 10, W], BF16, name=f"x0_{slab}", tag="bufA",
            padded_shape=[P, 3, 10, 10, SLAB]))

    # slab0's stage0 runs up front (nothing to hide it behind)
    for t in stage0_tasks(0, SLABS[0][0], SLABS[0][1], x0_tiles[0]):
        t()

    for slab, (b0, W) in enumerate(SLABS):
        x0s = x0_tiles[slab]
        # next slab's stage0 pairs, spread across this slab's conv1 groups
        if slab + 1 < len(SLABS):
            nb0, nW = SLABS[slab + 1]
            pending = stage0_tasks(slab + 1, nb0, nW, x0_tiles[slab + 1])
        else:
            pending = []
        n_groups = 30
        gi = 0

        # conv1: 384->384 3x3 pad1 on 10x10, batch slab of W.
        # psum = one output row [co, 10, W]; offsets boundary-split.
        for cot in range(3):
            w_t = w1_tiles[cot]
            for r in range(10):
                # spread next-slab stage0 pairs over the first 25 groups:
                # early enough that staging slots recycle for the slab
                # after, late enough that the staging DMAs keep up
                n_due = min(len(pending), ((gi + 1) * len(pending) + 24) // 25)
                n_done = min(len(pending), (gi * len(pending) + 24) // 25)
                for k in range(n_done, n_due):
                    pending[k]()
                gi += 1
                pt = pspool.tile([P, 10, W], FP32,
                                 name=f"p1_{slab}_{cot}_{r}", tag="ps",
                                 padded_shape=[P, 10, SLAB])
                first = True
                for cit in range(3):
                    for dh in (-1, 0, 1):
                        ir = r + dh
                        if ir < 0 or ir > 9:
                            continue
                        for dw in (-1, 0, 1):
                            ow0, own = max(0, -dw), min(10, 10 - dw)
                            iw0 = ow0 + dw
                            nw = own - ow0
                            nc.tensor.matmul(
                                pt[:, ow0:own, :],
                                lhsT=w_t[:, cit, dh + 1, dw + 1, :],
                                rhs=x0s[:, cit, ir, iw0 : iw0 + nw, :],
                                start=first,
                                stop=False,
                                skip_group_check=True,
                            )
                            first = False
                nc.scalar.activation(
                    x1[:, cot, r, :, b0 : b0 + W], pt[:], RELU,
                    bias=bias_t[:, cot : cot + 1],
                )

    w4_prefetch(3)  # reuses the 3 w1 slots as conv1 drains

    # ---- conv2: 384->384 3x3 stride2 pad1, 10x10 -> 5x5 ----
    x2 = pool_a.tile([P, 3, 5, 5, BL], BF16, name="x2", tag="bufA")
    for cot in range(3):
        w_t = w2_tiles[cot]
        for r in range(5):
            pt = pspool.tile([P, 5, BL], FP32, name=f"p2_{cot}_{r}",
                             tag="ps")
            first = True
            for cit in range(3):
                for dh in (-1, 0, 1):
                    ir = 2 * r + dh
                    if ir < 0 or ir > 9:
                        continue
                    for dw in (-1, 0, 1):
                        ow0 = 1 if dw == -1 else 0
                        nw = 5 - ow0
                        iw0 = 2 * ow0 + dw
                        nc.tensor.matmul(
                            pt[:, ow0:5, :],
                            lhsT=w_t[:, cit, dh + 1, dw + 1, :],
                            rhs=x1[:, cit, ir,
                                   iw0 : iw0 + 2 * nw - 1 : 2, :],
                            start=first,
                            stop=False,
                            skip_group_check=True,
                        )
                        first = False
            nc.scalar.activation(
                x2[:, cot, r, :, :], pt[:], RELU,
                bias=bias_t[:, 3 + cot : 4 + cot],
            )

    w4_prefetch(3)  # reuses the 3 w2 slots

    # ---- conv3: 384->512 3x3 pad1 on 5x5 ----
    x3 = pool_b.tile([P, 4, 5, 5, BL], BF16, name="x3", tag="bufB")
    for cot in range(4):
        w_t = w3_tiles[cot]
        for r in range(5):
            pt = pspool.tile([P, 5, BL], FP32, name=f"p3_{cot}_{r}",
                             tag="ps")
            first = True
            for cit in range(3):
                for dh in (-1, 0, 1):
                    ir = r + dh
                    if ir < 0 or ir > 4:
                        continue
                    for dw in (-1, 0, 1):
                        ow0, own = max(0, -dw), min(5, 5 - dw)
                        iw0 = ow0 + dw
                        nw = own - ow0
                        nc.tensor.matmul(
                            pt[:, ow0:own, :],
                            lhsT=w_t[:, cit, dh + 1, dw + 1, :],
                            rhs=x2[:, cit, ir, iw0 : iw0 + nw, :],
                            start=first,
                            stop=False,
                            skip_group_check=True,
                        )
                        first = False
            nc.scalar.activation(
                x3[:, cot, r, :, :], pt[:], RELU,
                bias=bias_t[:, 6 + cot : 7 + cot],
            )

    w4_prefetch(4)  # reuses the 4 w3 slots
    w4_prefetch(1)  # 16th tile rides the slot conv4's first tile frees

    # ---- conv4: 512->512 5x5 valid, 5x5 -> 1x1 (+ linear 512->10) ----
    x4 = misc.tile([P, 4, BL], BF16, name="x4")
    pl = pslin.tile([BL, 10], FP32, name="pl")
    for cot in range(4):
        pt = pspool.tile([P, BL], FP32, name=f"p4_{cot}", tag="ps")
        first = True
        for cit in range(4):
            w_t = w4_tiles[(cot, cit)]
            for rr in range(5):
                for cc in range(5):
                    nc.tensor.matmul(
                        pt[:],
                        lhsT=w_t[:, rr, cc, :],
                        rhs=x3[:, cit, rr, cc, :],
                        start=first,
                        stop=(cit == 3 and rr == 4 and cc == 4),
                    )
                    first = False
        nc.scalar.activation(
            x4[:, cot, :], pt[:], RELU,
            bias=bias_t[:, 10 + cot : 11 + cot],
        )
        # linear accumulation interleaved right behind each x4 eviction
        nc.tensor.matmul(
            pl[:],
            lhsT=x4[:, cot, :],
            rhs=wl_t[:, cot, :],
            start=(cot == 0),
            stop=(cot == 3),
        )
    out_sb = misc.tile([BL, 10], FP32, name="out_sb")
    nc.vector.tensor_add(out_sb[:], pl[:], bl_t[:])
    if accum_out:
        # accumulate into DRAM so R loop iterations yield R*result -
        # verifies the in-NEFF timing loop actually executed R times
        nc.gpsimd.dma_start(out_d[:], out_sb[:],
                            accum_op=mybir.AluOpType.add)
    else:
        nc.sync.dma_start(out_d[:], out_sb[:])


def _prep_core_inputs(A, S, W1, b1, W2, b2, W3, b3, W4, b4, Wl, bl):
    """Host-side reshapes (pure data movement). Returns list of per-core dicts."""
    B = A.shape[0]
    # stage0 block-diagonal stationary from A:
    # abd[b, hw, g*4+c, n*16+g] = A[b, n*64+g*4+c, h, w]
    A_r = np.ascontiguousarray(
        A.reshape(B, 8, 16, 4, 4).transpose(0, 4, 2, 3, 1)
    )  # [B, hw, g, c, n]
    abd = np.zeros((B, 4, 64, P), dtype=ml_dtypes.bfloat16)
    abd_v = abd.reshape(B, 4, 16, 4, 8, 16)  # [B, hw, g, c, n, g']
    gg = np.arange(16)
    abd_v[:, :, gg, :, :, gg] = A_r.transpose(2, 0, 1, 3, 4)[gg].astype(
        ml_dtypes.bfloat16
    )
    # hw-pair staging: stationary rows stack (w=0 block over w=1 block) of
    # the 64-row gc contraction; j-th pair's stationary in cols j*128..;
    # the moving S is block-diagonal over the two w positions.
    abd2 = abd.reshape(B, 2, 2, 64, P).transpose(0, 2, 3, 1, 4).reshape(
        B, P, 2 * P)
    # S block-diagonal with columns permuted to (o, p, w, q): the matmul
    # output then lands directly in the (o, row, col) psum image order.
    s2v = S.reshape(B, 64, 3, 5, 5).astype(ml_dtypes.bfloat16)
    sdiag6 = np.zeros((B, P, 3, 5, 2, 5), dtype=ml_dtypes.bfloat16)
    sdiag6[:, 0:64, :, :, 0, :] = s2v
    sdiag6[:, 64:128, :, :, 1, :] = s2v
    sdiag = sdiag6.reshape(B, P, 150)
    stg = np.concatenate([abd2, sdiag], axis=2)
    stg = np.ascontiguousarray(
        stg.reshape(B // SG0, SG0, P, 2 * P + 150).transpose(0, 2, 1, 3))

    def conv_w_tiles(W, n_cot, n_cit, k):
        co, ci = W.shape[0], W.shape[1]
        # -> [cot, cip, cit, kh, kw, cof]
        t = W.reshape(n_cot, P, n_cit, P, k, k).transpose(0, 3, 2, 4, 5, 1)
        return np.ascontiguousarray(t).astype(ml_dtypes.bfloat16)

    # x0's channel layout from stage0 is (o, n*16+g); permute W1's ci to match
    # (reference ci index = n*48 + g*3 + o).
    o_i, n_i, g_i = np.meshgrid(
        np.arange(3), np.arange(8), np.arange(16), indexing="ij"
    )
    perm = (n_i * 48 + g_i * 3 + o_i).reshape(-1)
    w1t = conv_w_tiles(W1[:, perm], 3, 3, 3)
    w2t = conv_w_tiles(W2, 3, 3, 3)
    w3t = conv_w_tiles(W3, 4, 3, 3)
    w4t = conv_w_tiles(W4, 4, 4, 5)

    wl_a = np.ascontiguousarray(
        Wl.T.reshape(4, P, 10).transpose(1, 0, 2)
    ).astype(ml_dtypes.bfloat16)
    biases = np.zeros((P, 14), np.float32)
    biases[:, 0:3] = b1.reshape(3, P).T
    biases[:, 3:6] = b2.reshape(3, P).T
    biases[:, 6:10] = b3.reshape(4, P).T
    biases[:, 10:14] = b4.reshape(4, P).T
    blrep = np.tile(bl.astype(np.float32), (BL, 1))

    in_maps = []
    for c in range(NCORES):
        sl = slice(c * (BL // SG0), (c + 1) * (BL // SG0))
        in_maps.append({
            "stg": stg[sl],
            "w1t": w1t, "w2t": w2t, "w3t": w3t, "w4t": w4t,
            "wl": wl_a, "biases": biases, "blrep": blrep,
        })
    return in_maps


_PROGRAM_CACHE = {}


def _get_program():
    if "nc" not in _PROGRAM_CACHE:
        _PROGRAM_CACHE["nc"] = _build_program()
    return _PROGRAM_CACHE["nc"]


def kernel(A, S, W1, b1, W2, b2, W3, b3, W4, b4, Wl, bl):
    A = np.asarray(A, np.float32)
    S = np.asarray(S, np.float32)
    in_maps = _prep_core_inputs(
        A, S,
        np.asarray(W1, np.float32), np.asarray(b1, np.float32),
        np.asarray(W2, np.float32), np.asarray(b2, np.float32),
        np.asarray(W3, np.float32), np.asarray(b3, np.float32),
        np.asarray(W4, np.float32), np.asarray(b4, np.float32),
        np.asarray(Wl, np.float32), np.asarray(bl, np.float32),
    )
    nc = _get_program()
    res = run_bass_kernel_spmd(nc, in_maps, list(range(NCORES)))
    return np.concatenate([res.results[c]["OUT"] for c in range(NCORES)], axis=0)

